# revision 1
# baseline (speedup 1.0000x reference)
"""Trainium2 Bass kernel for nn_ConvDatapath: quantized bit-sliced crossbar conv.

Pipeline (per core, data-parallel over Nx=6272 im2col rows, 784 rows/core):
  host: im2col (pure layout) -> xf [784, 580] per core (zero-padded K 576->580)
  device:
    1. per-row unsigned 8-bit quantization of x rows and w rows
       (min/max/sum reductions, q = rint((v-min)*inv) via the 2^23 magic-add
       trick fused into one ScalarE activation)
    2. PE-transpose of quantized (M+q) tiles into [K_block, rows] layout
    3. bit-slice into 4x 2-bit planes (int32 shift/and), convert to bf16
    4. 80 matmuls (5 K-blocks x 4 w-slices x 4 x-slices) [116]x[116,128]
       accumulating exact small-int products in PSUM f32
    5. ADC quantization 4*round(z/4) exactly via ScalarE activation
       Relu(z*(c/4) + c*M) with c = 4*WSF[ws]*ISF[is] (power of two), then
       DVE scalar_tensor_tensor (t - c*M) + acc accumulates the recombined
       integer Z exactly in f32 (|Z| < 2^24)
    6. dequant: Z*xs*ws + rank-2 offset correction via a tiny K=2 matmul
  host: gather per-core [128, 784] outputs -> [2,128,56,56]

All integer arithmetic is exact in f32; the only deviations from the jax
reference are sub-ulp rounding-tie differences in the quantizer ratio.
"""
import sys

sys.path.insert(0, "/opt/trn_rl_repo")

import numpy as np

# ---- problem constants (hardcoded per contract) ----
B, CIN, H, W_ = 2, 64, 56, 56
COUT, KH, KW = 128, 3, 3
K = CIN * KH * KW            # 576
NB, NPB = 5, 116             # chunker: 5 blocks of 116 (pad 4)
KPAD = NB * NPB              # 580
NCORES = 8
NX = B * H * W_              # 6272
R = NX // NCORES             # 784 rows per core
RT = 112                     # row tile -> 7 tiles per core
NJ = R // RT                 # 7
MAGIC = float(2 ** 23)
WSF = [64.0, 16.0, 4.0, 1.0]
ISF = [64.0, 16.0, 4.0, 1.0]
SH = [6, 4, 2, 0]            # slice shifts

_NC_CACHE = {}


def _build_program():
    import concourse.bass as bass
    import concourse.bacc as bacc
    import concourse.tile as tile
    from concourse import mybir
    from concourse.masks import make_identity

    f32 = mybir.dt.float32
    i32 = mybir.dt.int32
    bf16 = mybir.dt.bfloat16
    AF = mybir.ActivationFunctionType
    OP = mybir.AluOpType
    AX = mybir.AxisListType

    nc = bacc.Bacc("TRN2", target_bir_lowering=False, debug=False)

    d_xf = nc.dram_tensor("xf", (R, KPAD), f32, kind="ExternalInput")
    d_wf = nc.dram_tensor("wf", (COUT, KPAD), f32, kind="ExternalInput")
    d_out = nc.dram_tensor("out", (COUT, R), f32, kind="ExternalOutput")

    with tile.TileContext(nc) as tc:
        with (
            tc.tile_pool(name="const", bufs=1) as cpool,
            tc.tile_pool(name="work", bufs=2) as work,
            tc.tile_pool(name="stage", bufs=4) as stage,
            tc.tile_pool(name="psum", bufs=2, space="PSUM") as pps,
            tc.tile_pool(name="psz", bufs=3, space="PSUM") as psz,
        ):
            ident = cpool.tile([128, 128], f32)
            make_identity(nc, ident[:])

            # per-(ws,is) ADC bias constants c*M
            biasMC = cpool.tile([128, 16], f32)
            for wsi in range(4):
                for isi in range(4):
                    c = 4.0 * WSF[wsi] * ISF[isi]
                    nc.vector.memset(biasMC[:, wsi * 4 + isi : wsi * 4 + isi + 1], c * MAGIC)

            # ---------------- W prep ----------------
            w_sb = work.tile([COUT, KPAD], f32)
            nc.sync.dma_start(w_sb[:], d_wf.ap())
            w_min = cpool.tile([COUT, 1], f32)
            w_max = work.tile([COUT, 1], f32)
            w_sum = work.tile([COUT, 1], f32)
            nc.vector.tensor_reduce(w_min[:], w_sb[:], axis=AX.X, op=OP.min)
            nc.vector.tensor_reduce(w_max[:], w_sb[:], axis=AX.X, op=OP.max)
            nc.vector.tensor_reduce(w_sum[:], w_sb[:], axis=AX.X, op=OP.add)
            w_scale = cpool.tile([COUT, 1], f32)
            w_rng = work.tile([COUT, 1], f32)
            nc.vector.tensor_tensor(w_rng[:], w_max[:], w_min[:], op=OP.subtract)
            nc.vector.tensor_scalar(w_scale[:], w_rng[:], float(np.float32(1.0/255.0)), None, op0=OP.mult)
            w_inv = cpool.tile([COUT, 1], f32)
            nc.vector.reciprocal(w_inv[:], w_scale[:])
            Mtile = cpool.tile([128, 1], f32)
            nc.vector.memset(Mtile[:], MAGIC)
            w_negmin = work.tile([COUT, 1], f32)
            nc.vector.tensor_scalar(w_negmin[:], w_min[:], -1.0, None, op0=OP.mult)
            w_vr = work.tile([COUT, KPAD], f32)
            nc.scalar.activation(w_vr[:], w_sb[:], AF.Relu, bias=w_negmin[:], scale=1.0)

            qMw = work.tile([COUT, KPAD], f32)
            nc.scalar.activation(qMw[:], w_vr[:], AF.Relu, bias=Mtile[:], scale=w_inv[:])
            nc.vector.memset(qMw[:, K:KPAD], MAGIC)

            # wsl[b][ws]: [116, 128] bf16 stationary operands
            wslb = [[cpool.tile([NPB, COUT], bf16, tag=f"wsl{b}_{s}", name=f"wsl{b}_{s}") for s in range(4)]
                    for b in range(NB)]
            for b in range(NB):
                ps_t = pps.tile([NPB, COUT], f32, tag="ps_tr")
                nc.tensor.transpose(ps_t[:], qMw[:, b * NPB:(b + 1) * NPB], ident[:])
                wQT = work.tile([NPB, COUT], f32, tag="wQT")
                nc.scalar.copy(wQT[:], ps_t[:])
                wqi = wQT[:].bitcast(i32)
                for s in range(4):
                    wsl_i = work.tile([NPB, COUT], i32, tag="wsl_i")
                    if SH[s]:
                        nc.vector.tensor_scalar(wsl_i[:], wqi, SH[s], 3,
                                                op0=OP.logical_shift_right, op1=OP.bitwise_and)
                    else:
                        nc.vector.tensor_scalar(wsl_i[:], wqi, 3, None, op0=OP.bitwise_and)
                    nc.vector.tensor_copy(wslb[b][s][:], wsl_i[:])

            # correction row vectors: U1 = w_sum - 576*w_min ; U2 = w_min
            Upair = work.tile([COUT, 2], f32)
            nc.vector.scalar_tensor_tensor(Upair[:, 0:1], w_min[:], -576.0, w_sum[:],
                                           op0=OP.mult, op1=OP.add)
            nc.vector.tensor_copy(Upair[:, 1:2], w_min[:])
            ps_u = pps.tile([2, COUT], f32, tag="ps_tr")
            nc.tensor.transpose(ps_u[:], Upair[:], ident[:])
            UT = cpool.tile([2, COUT], f32)
            nc.scalar.copy(UT[:], ps_u[:])

            # ---------------- X prep ----------------
            QTx = [cpool.tile([NPB, R], f32, tag=f"QTx{b}", name=f"QTx{b}") for b in range(NB)]
            Vrow = cpool.tile([2, R], f32)   # rows: x_min, x_sum
            Vxs = cpool.tile([1, R], f32)    # x_scale row

            for j in range(NJ):
                x_sb = stage.tile([RT, KPAD], f32, tag="x_sb")
                nc.sync.dma_start(x_sb[:], d_xf.ap()[j * RT:(j + 1) * RT, :])
                xmin = stage.tile([RT, 1], f32, tag="xmin")
                xmax = stage.tile([RT, 1], f32, tag="xmax")
                xsum = stage.tile([RT, 1], f32, tag="xsum")
                nc.vector.tensor_reduce(xmin[:], x_sb[:], axis=AX.X, op=OP.min)
                nc.vector.tensor_reduce(xmax[:], x_sb[:], axis=AX.X, op=OP.max)
                nc.vector.tensor_reduce(xsum[:], x_sb[:], axis=AX.X, op=OP.add)
                xrng = stage.tile([RT, 1], f32, tag="xrng")
                nc.vector.tensor_tensor(xrng[:], xmax[:], xmin[:], op=OP.subtract)
                xscale = stage.tile([RT, 1], f32, tag="xscale")
                nc.vector.tensor_scalar(xscale[:], xrng[:], float(np.float32(1.0/255.0)), None, op0=OP.mult)
                xinv = stage.tile([RT, 1], f32, tag="xinv")
                nc.vector.reciprocal(xinv[:], xscale[:])
                xnegmin = stage.tile([RT, 1], f32, tag="xnegmin")
                nc.vector.tensor_scalar(xnegmin[:], xmin[:], -1.0, None, op0=OP.mult)
                x_vr = stage.tile([RT, KPAD], f32, tag="x_vr")
                nc.scalar.activation(x_vr[:], x_sb[:], AF.Relu, bias=xnegmin[:], scale=1.0)

                qMx = stage.tile([RT, KPAD], f32, tag="qMx")
                nc.scalar.activation(qMx[:], x_vr[:], AF.Relu, bias=Mtile[:RT], scale=xinv[:])
                nc.vector.memset(qMx[:, K:KPAD], MAGIC)

                # stats triple -> V rows via transpose
                Vtri = stage.tile([RT, 2], f32, tag="Vtri")
                nc.vector.tensor_copy(Vtri[:, 0:1], xmin[:])
                nc.vector.tensor_copy(Vtri[:, 1:2], xsum[:])
                ps_v = pps.tile([2, RT], f32, tag="ps_tr")
                nc.tensor.transpose(ps_v[:], Vtri[:], ident[:RT, :RT])
                nc.scalar.copy(Vrow[:, j * RT:(j + 1) * RT], ps_v[:])
                ps_x = pps.tile([1, RT], f32, tag="ps_tr")
                nc.tensor.transpose(ps_x[:], xscale[:], ident[:RT, :RT])
                nc.scalar.copy(Vxs[:, j * RT:(j + 1) * RT], ps_x[:])

                for b in range(NB):
                    ps_q = pps.tile([NPB, RT], f32, tag="ps_tr")
                    nc.tensor.transpose(ps_q[:], qMx[:, b * NPB:(b + 1) * NPB], ident[:RT, :RT])
                    nc.scalar.copy(QTx[b][:, j * RT:(j + 1) * RT], ps_q[:])

            # bit-slice planes, bf16
            xslb = [[cpool.tile([NPB, R], bf16, tag=f"xsl{b}_{s}", name=f"xsl{b}_{s}") for s in range(4)]
                    for b in range(NB)]
            for b in range(NB):
                xqi = QTx[b][:].bitcast(i32)
                for s in range(4):
                    xsl_i = work.tile([NPB, R], i32, tag="xsl_i")
                    if SH[s]:
                        nc.vector.tensor_scalar(xsl_i[:], xqi, SH[s], 3,
                                                op0=OP.logical_shift_right, op1=OP.bitwise_and)
                    else:
                        nc.vector.tensor_scalar(xsl_i[:], xqi, 3, None, op0=OP.bitwise_and)
                    if s % 2 == 0:
                        nc.vector.tensor_copy(xslb[b][s][:], xsl_i[:])
                    else:
                        nc.vector.tensor_copy(xslb[b][s][:], xsl_i[:])

            # ---------------- main loop ----------------
            out_t = cpool.tile([COUT, R], f32)
            HR = R // 2  # 392
            first = True
            for b in range(NB):
                for wsi in range(4):
                    for isi in range(4):
                        zps = psz.tile([128, 2, 512], f32, tag="zps")
                        nc.tensor.matmul(zps[:, 0, :HR], wslb[b][wsi][:],
                                         xslb[b][isi][:, 0:HR], start=True, stop=True)
                        nc.tensor.matmul(zps[:, 1, :HR], wslb[b][wsi][:],
                                         xslb[b][isi][:, HR:R], start=True, stop=True)
                        c = 4.0 * WSF[wsi] * ISF[isi]
                        tst = stage.tile([COUT, R], f32, tag="tst")
                        tst3 = tst[:].rearrange("p (a n) -> p a n", a=2)
                        nc.scalar.activation(tst3, zps[:, :, :HR], AF.Relu,
                                             bias=biasMC[:, wsi * 4 + isi: wsi * 4 + isi + 1],
                                             scale=c / 4.0)
                        if first:
                            nc.vector.tensor_scalar(out_t[:], tst[:], c * MAGIC, None,
                                                    op0=OP.subtract)
                            first = False
                        else:
                            eng = nc.vector
                            eng.scalar_tensor_tensor(out_t[:], tst[:], c * MAGIC, out_t[:],
                                                     op0=OP.subtract, op1=OP.add)

            # ---------------- dequant + corrections ----------------
            cps = psz.tile([128, 2, 512], f32, tag="zps")
            nc.tensor.matmul(cps[:, 0, :HR], UT[:], Vrow[0:2, 0:HR], start=True, stop=True)
            nc.tensor.matmul(cps[:, 1, :HR], UT[:], Vrow[0:2, HR:R], start=True, stop=True)

            # xs broadcast along partitions via ones-outer-product
            ones1 = cpool.tile([1, COUT], f32)
            nc.vector.memset(ones1[:], 1.0)
            xs_ps = psz.tile([128, 2, 512], f32, tag="zps")
            nc.tensor.matmul(xs_ps[:, 0, :HR], ones1[:], Vxs[:, 0:HR], start=True, stop=True)
            nc.tensor.matmul(xs_ps[:, 1, :HR], ones1[:], Vxs[:, HR:R], start=True, stop=True)

            outf = work.tile([COUT, R], f32)
            outf3 = outf[:].rearrange("p (a n) -> p a n", a=2)
            out_t3 = out_t[:].rearrange("p (a n) -> p a n", a=2)
            nc.vector.scalar_tensor_tensor(outf3, out_t3, w_scale[:], xs_ps[:, :, :HR],
                                           op0=OP.mult, op1=OP.mult)
            nc.vector.tensor_tensor(outf3, outf3, cps[:, :, :HR], op=OP.add)
            nc.sync.dma_start(d_out.ap(), outf[:])

    nc.compile()
    return nc


def _get_nc():
    if "nc" not in _NC_CACHE:
        _NC_CACHE["nc"] = _build_program()
    return _NC_CACHE["nc"]


def _im2col_host(x):
    # 3x3 SAME patches, column order [Cin, kh, kw]; rows (b, h, w)
    xp = np.pad(x, ((0, 0), (0, 0), (1, 1), (1, 1)))  # [B, C, 58, 58]
    s = xp.strides
    v = np.lib.stride_tricks.as_strided(
        xp,
        shape=(B, H, W_, CIN, KH, KW),
        strides=(s[0], s[2], s[3], s[1], s[2], s[3]),
    )
    return v.reshape(NX, K)


def kernel(x, w):
    from concourse.bass_utils import run_bass_kernel_spmd

    nc = _get_nc()
    x = np.ascontiguousarray(np.asarray(x, dtype=np.float32))
    w = np.asarray(w, dtype=np.float32)

    xf = np.zeros((NX, KPAD), np.float32)
    xf[:, :K] = _im2col_host(x)
    wf = np.zeros((COUT, KPAD), np.float32)
    wf[:, :K] = w.reshape(COUT, K)

    in_maps = [{"xf": np.ascontiguousarray(xf[c * R:(c + 1) * R]), "wf": wf}
               for c in range(NCORES)]
    import os
    trace = bool(os.environ.get("CONV_KERNEL_TRACE"))
    try:
        res = run_bass_kernel_spmd(nc, in_maps, core_ids=list(range(NCORES)), trace=trace)
    except Exception:
        if not trace:
            raise
        res = run_bass_kernel_spmd(nc, in_maps, core_ids=list(range(NCORES)), trace=False)
    _NC_CACHE["last_results"] = res
    z = np.concatenate([res.results[c]["out"].T for c in range(NCORES)], axis=0)
    return np.ascontiguousarray(
        z.reshape(B, H, W_, COUT).transpose(0, 3, 1, 2).astype(np.float32))



# revision 12
# speedup vs baseline: 1.3119x; 1.3119x over previous
"""Trainium2 Bass kernel for nn_ConvDatapath: quantized bit-sliced crossbar conv.

v2 pipeline (per core, data-parallel over Nx=6272 im2col rows, 784 rows/core):
  host: im2col (layout only) -> xf [784, 580] per core (K 576 zero-padded to 580)
  device:
    prep (per 128-row tile j):
      - DVE min/max row reductions; Pool computes sum(x-min) via
        tensor_scalar accum_out; small per-row scale/bias ops
      - one ScalarE activation quantizes: qM = Relu(x*inv + (M - min*inv))
        (magic 2^23 add rounds to integer exactly)
      - ScalarE copies qM -> q as uint16 into a block-padded [128,5,128] tile
      - DMA-engine xbar transpose ([128,5,128] -> [128,5,128] per-block
        transposed) builds XQT [k, b, row] with zero engine cost
    slicing (per k-block b, pipelined with main loop):
      - 4 DVE u16 shift/and ops extract 2-bit planes (2x DVE mode)
      - 1 DVE copy converts planes to bf16 for the PE
    main loop (80 iterations = 5 blocks x 4 w-slices x 4 x-slices):
      - 2 bf16 matmuls [116]x[116,392] -> PSUM (exact small-int products)
      - ADC round: T = fp16(z*0.25 + 1024) on ScalarE or DVE; the f32->fp16
        convert rounds to integer exactly (values in [1024,2048) have ulp 1)
      - accumulate: PE matmul acc += (c*I)^T @ T into a persistent PSUM
        accumulator (fp16 x fp16 products exact in f32), a few on DVE
    tail: rank-3 correction matmul (offset terms + 1024-bias removal) and
      two DVE ops produce the dequantized output; DMA out [128, 784]
  host: gather per-core [128, 784] outputs -> [2,128,56,56]
"""
import sys

sys.path.insert(0, "/opt/trn_rl_repo")

import numpy as np

# ---- problem constants (hardcoded per contract) ----
B, CIN, H, W_ = 2, 64, 56, 56
COUT, KH, KW = 128, 3, 3
K = CIN * KH * KW            # 576
NB, NPB = 5, 116             # chunker: 5 blocks of 116 (pad 4)
KPAD = NB * NPB              # 580
NCORES = 8
NX = B * H * W_              # 6272
R = NX // NCORES             # 784 rows per core
RT = 128                     # row tile (xbar-aligned) -> 7 tiles, last has 16
NJ = 7
RPADT = NJ * RT              # 896 padded row count in transposed tiles
MAGIC = float(2 ** 23)
WSF = [64.0, 16.0, 4.0, 1.0]
ISF = [64.0, 16.0, 4.0, 1.0]
SH = [6, 4, 2, 0]            # slice shifts
HR = R // 2                  # 392
# c for iteration (ws, is); ADC out contributes c*round(z/4) with c=4*WSF*ISF
C_OF = [[4.0 * WSF[w] * ISF[i] for i in range(4)] for w in range(4)]
C_VALUES = sorted({C_OF[w][i] for w in range(4) for i in range(4)}, reverse=True)
C0 = 1024.0 * NB * sum(C_OF[w][i] for w in range(4) for i in range(4))

# scheduling knobs
ROUND_ON_S = [i % 4 != 3 for i in range(80)]     # 60 ScalarE / 20 DVE rounds
ACC_ON_D = [False for i in range(80)]            # all-PE accumulates (debug)

_NC_CACHE = {}


def _build_program():
    import concourse.bass as bass
    import concourse.bacc as bacc
    import concourse.tile as tile
    from concourse import mybir
    from concourse.masks import make_identity

    f32 = mybir.dt.float32
    u16 = mybir.dt.uint16
    bf16 = mybir.dt.bfloat16
    fp16 = mybir.dt.float16
    AF = mybir.ActivationFunctionType
    OP = mybir.AluOpType
    AX = mybir.AxisListType
    INV255 = float(np.float32(1.0 / 255.0))

    import os
    dbg = bool(os.environ.get("CONV_DEBUG"))
    nc = bacc.Bacc("TRN2", target_bir_lowering=False, debug=False)

    d_xf = nc.dram_tensor("xf", (R, KPAD), f32, kind="ExternalInput")
    d_wf = nc.dram_tensor("wf", (COUT, KPAD), f32, kind="ExternalInput")
    d_out = nc.dram_tensor("out", (COUT, R), f32, kind="ExternalOutput")
    if dbg:
        d_dxqt = nc.dram_tensor("dbg_xqt", (128, NJ, NB, 128), mybir.dt.uint16,
                                kind="ExternalOutput")
        d_dwsl = nc.dram_tensor("dbg_wsl", (128, 4, NB, 128), f32,
                                kind="ExternalOutput")
        d_dvrow = nc.dram_tensor("dbg_vrow", (3, RPADT), f32,
                                 kind="ExternalOutput")
        d_dacc = nc.dram_tensor("dbg_acc", (COUT, R), f32,
                                kind="ExternalOutput")

    with tile.TileContext(nc) as tc:
        with (
            tc.tile_pool(name="const", bufs=1) as cpool,
            tc.tile_pool(name="stage", bufs=2) as stage,
            tc.tile_pool(name="tpool", bufs=4) as tpool,
            tc.tile_pool(name="psz", bufs=3, space="PSUM") as psz,
            tc.tile_pool(name="pacc", bufs=1, space="PSUM") as pacc,
        ):
            ident = cpool.tile([128, 128], f32)
            make_identity(nc, ident[:])
            Mtile = cpool.tile([128, 1], f32)
            nc.vector.memset(Mtile[:], MAGIC)
            Kbias = cpool.tile([128, 1], f32)
            nc.vector.memset(Kbias[:], 1024.0)
            ones1 = cpool.tile([1, COUT], f32)
            nc.vector.memset(ones1[:], 1.0)

            # c * identity fp16 stationary tiles for the accumulate matmuls
            cI = {}
            for cv in C_VALUES:
                t = cpool.tile([128, 128], fp16, name=f"cI{int(cv)}")
                nc.gpsimd.memset(t[:], 0.0)
                nc.gpsimd.affine_select(out=t[:], in_=t[:],
                                        compare_op=OP.not_equal, fill=cv,
                                        base=0, pattern=[[-1, 128]],
                                        channel_multiplier=1)
                cI[cv] = t

            # ---------------- W prep ----------------
            w_sb = cpool.tile([COUT, KPAD], f32)
            nc.sync.dma_start(w_sb[:], d_wf.ap())
            w_min = cpool.tile([COUT, 1], f32)
            w_max = stage.tile([COUT, 1], f32, tag="wmax")
            nc.vector.tensor_reduce(w_min[:], w_sb[:], axis=AX.X, op=OP.min)
            nc.vector.tensor_reduce(w_max[:], w_sb[:], axis=AX.X, op=OP.max)
            w_scale = cpool.tile([COUT, 1], f32)
            # w_scale = (max - min)/255
            nc.vector.scalar_tensor_tensor(w_scale[:], w_max[:], 1.0, w_min[:],
                                           op0=OP.mult, op1=OP.subtract)
            nc.vector.tensor_scalar(w_scale[:], w_scale[:], INV255, None, op0=OP.mult)
            w_inv = cpool.tile([COUT, 1], f32)
            nc.vector.reciprocal(w_inv[:], w_scale[:])
            # vr = w - min (rounded f32, matching the reference), accum -> A_w
            w_vr = stage.tile([COUT, KPAD], f32, tag="wvr")
            w_accA = cpool.tile([COUT, 1], f32)
            nc.vector.tensor_scalar(w_vr[:], w_sb[:], w_min[:], 0.0,
                                    op0=OP.subtract, op1=OP.add,
                                    accum_out=w_accA[:])
            # quantize: qMw = Relu(vr*inv + M) = M + qw
            qMw = stage.tile([COUT, KPAD], f32, tag="qMw")
            nc.scalar.activation(qMw[:], w_vr[:], AF.Relu, bias=Mtile[:],
                                 scale=w_inv[:])
            # u16ify into block-padded layout
            wq_u16 = stage.tile([COUT, NB, 128], u16, tag="wqu")
            nc.scalar.activation(
                wq_u16[:, :, 0:NPB],
                qMw[:].rearrange("p (b c) -> p b c", b=NB),
                AF.Copy, bias=-MAGIC)
            # xbar transpose: WQT[k, b, cout]
            WQT = cpool.tile([128, NB, 128], u16)
            nc.sync.dma_start_transpose(
                WQT[:], wq_u16[:].rearrange("p b c -> p (b c)"))
            # slices: WSLu[k, s, b, cout] -> bf16
            WSLu = stage.tile([128, 4, NB, 128], u16, tag="wslu")
            for s in range(4):
                if SH[s]:
                    nc.vector.tensor_scalar(WSLu[:, s], WQT[:], SH[s], 3,
                                            op0=OP.logical_shift_right,
                                            op1=OP.bitwise_and)
                else:
                    nc.vector.tensor_scalar(WSLu[:, s], WQT[:], 3, None,
                                            op0=OP.bitwise_and)
            WSL = cpool.tile([128, 4, NB, 128], bf16)
            nc.vector.tensor_copy(WSL[:], WSLu[:])

            # correction row operands:
            # U1' = A_w + 580*w_min (pairs x_min)
            # U2  = w_min           (pairs A_x = sum(x-min) over 580)
            # U3  = -C0*w_scale     (pairs x_scale; removes 1024 round bias)
            Upair = stage.tile([COUT, 3], f32, tag="upair")
            nc.vector.tensor_scalar(Upair[:, 0:1], w_scale[:], -C0, None,
                                    op0=OP.mult)
            nc.vector.scalar_tensor_tensor(Upair[:, 1:2], w_min[:], 584.0,
                                           w_accA[:], op0=OP.mult, op1=OP.add)
            nc.vector.tensor_copy(Upair[:, 2:3], w_min[:])
            ps_u = psz.tile([128, 2, 512], f32, tag="zps")
            nc.tensor.transpose(ps_u[0:3, 0, 0:COUT], Upair[:], ident[:])
            UT = cpool.tile([3, COUT], f32)
            nc.scalar.copy(UT[:], ps_u[0:3, 0, 0:COUT])

            # ---------------- X prep ----------------
            XQT = cpool.tile([128, NJ, NB, 128], u16)   # [k, j, b, row_in_j]
            Vrow = cpool.tile([3, RPADT], f32)          # rows: xscale, xmin, A_x

            for j in range(NJ):
                nr = RT if j < NJ - 1 else R - RT * (NJ - 1)   # 128 or 16
                x_sb = stage.tile([RT, KPAD], f32, tag="x_sb")
                nc.sync.dma_start(x_sb[0:nr, :], d_xf.ap()[j * RT:j * RT + nr, :])
                Vtri = stage.tile([RT, 3], f32, tag="vtri")
                xmax = stage.tile([RT, 1], f32, tag="xmax")
                nc.vector.tensor_reduce(Vtri[:, 1:2], x_sb[:], axis=AX.X, op=OP.min)
                nc.vector.tensor_reduce(xmax[:], x_sb[:], axis=AX.X, op=OP.max)
                # xscale = (max-min)/255 into Vtri col 0 (Pool)
                nc.gpsimd.tensor_tensor(Vtri[:, 0:1], xmax[:], Vtri[:, 1:2],
                                        op=OP.subtract)
                nc.gpsimd.tensor_scalar(Vtri[:, 0:1], Vtri[:, 0:1], INV255, None,
                                        op0=OP.mult)
                xinv = stage.tile([RT, 1], f32, tag="xinv")
                nc.vector.reciprocal(xinv[:], Vtri[:, 0:1])
                # vr = x - min (rounded f32, matching reference), accum -> A_x
                vr_scr = stage.tile([RT, KPAD], f32, tag="vrscr")
                nc.vector.tensor_scalar(vr_scr[:], x_sb[:], Vtri[:, 1:2], 0.0,
                                        op0=OP.subtract, op1=OP.add,
                                        accum_out=Vtri[:, 2:3])
                # quantize
                qMx = stage.tile([RT, KPAD], f32, tag="qMx")
                nc.scalar.activation(qMx[:], vr_scr[:], AF.Relu, bias=Mtile[0:RT],
                                     scale=xinv[:])
                # u16ify block-padded
                qu16 = stage.tile([RT, NB, 128], u16, tag="qu16")
                nc.scalar.activation(
                    qu16[:, :, 0:NPB],
                    qMx[:].rearrange("p (b c) -> p b c", b=NB),
                    AF.Copy, bias=-MAGIC)
                # zero k=576..579 pad cols of block 4 before transposing
                nc.vector.memset(qu16[:, 4, 112:116], 0)
                # xbar transpose into XQT columns j*128..+128
                nc.sync.dma_start_transpose(
                    XQT[:, j],
                    qu16[:].rearrange("p b c -> p (b c)"))
                # stats transpose -> Vrow cols
                ps_v = psz.tile([128, 2, 512], f32, tag="zps")
                nc.tensor.transpose(ps_v[0:3, 0, 0:RT], Vtri[:], ident[0:RT, 0:RT])
                nc.scalar.copy(Vrow[:, j * RT:(j + 1) * RT], ps_v[0:3, 0, 0:RT])


            # ---------------- main loop with inline slicing ----------------
            acc = pacc.tile([128, 2, 512], f32)
            out_dve = cpool.tile([COUT, R], f32)
            out_dve3 = out_dve[:].rearrange("p (a n) -> p a n", a=2)
            n_acc_d = sum(1 for i in range(80) if ACC_ON_D[i])

            def emit_slice(b):
                xslu = tpool.tile([128, 4, RPADT], u16, tag="xslu")
                for s in range(4):
                    if SH[s]:
                        nc.vector.tensor_scalar(xslu[:, s], XQT[:, :, b, :],
                                                SH[s], 3,
                                                op0=OP.logical_shift_right,
                                                op1=OP.bitwise_and)
                    else:
                        nc.vector.tensor_scalar(xslu[:, s], XQT[:, :, b, :],
                                                3, None, op0=OP.bitwise_and)
                xsl = tpool.tile([128, 4, RPADT], bf16, tag="xsl")
                nc.vector.tensor_copy(xsl[:], xslu[:])
                return xsl

            xsl_cur = emit_slice(0)
            xsl_next = None
            pend = []           # (kind, payload) pending PE accumulate emission
            it = 0
            first_d = True
            n_pe_acc = 80 - n_acc_d
            pe_acc_done = 0
            for b in range(NB):
                for wsi in range(4):
                    for isi in range(4):
                        c = C_OF[wsi][isi]
                        zps = psz.tile([128, 2, 512], f32, tag="zps")
                        nc.tensor.matmul(zps[:, 0, 0:HR],
                                         WSL[0:NPB, wsi, b, :],
                                         xsl_cur[0:NPB, isi, 0:HR],
                                         start=True, stop=True)
                        nc.tensor.matmul(zps[:, 1, 0:HR],
                                         WSL[0:NPB, wsi, b, :],
                                         xsl_cur[0:NPB, isi, HR:R],
                                         start=True, stop=True)
                        # flush one pending accumulate (software pipeline lag 1)
                        if pend:
                            Tp, cp = pend.pop(0)
                            nc.tensor.matmul(acc[:, 0, 0:HR], cI[cp][:],
                                             Tp[:, 0, :], start=(pe_acc_done == 0),
                                             stop=False, skip_group_check=True)
                            nc.tensor.matmul(acc[:, 1, 0:HR], cI[cp][:],
                                             Tp[:, 1, :], start=(pe_acc_done == 0),
                                             stop=(pe_acc_done == n_pe_acc - 1),
                                             skip_group_check=True)
                            pe_acc_done += 1
                        # ADC round
                        T = tpool.tile([128, 2, HR], fp16, tag="T")
                        if ROUND_ON_S[it]:
                            nc.scalar.activation(T[:], zps[:, :, 0:HR], AF.Relu,
                                                 bias=Kbias[:], scale=0.25)
                        else:
                            nc.vector.tensor_scalar(T[:], zps[:, :, 0:HR],
                                                    0.25, 1024.0,
                                                    op0=OP.mult, op1=OP.add)
                        # accumulate
                        if ACC_ON_D[it]:
                            if first_d:
                                nc.vector.tensor_scalar(out_dve3, T[:], c, None,
                                                        op0=OP.mult)
                                first_d = False
                            else:
                                nc.vector.scalar_tensor_tensor(out_dve3, T[:], c,
                                                               out_dve3,
                                                               op0=OP.mult,
                                                               op1=OP.add)
                        else:
                            pend.append((T, c))
                        # interleave next block's slicing mid-block
                        if isi == 3 and wsi == 1 and b < NB - 1:
                            xsl_next = emit_slice(b + 1)
                        it += 1
                xsl_cur = xsl_next
            # flush remaining pending accumulates
            while pend:
                Tp, cp = pend.pop(0)
                nc.tensor.matmul(acc[:, 0, 0:HR], cI[cp][:], Tp[:, 0, :],
                                 start=(pe_acc_done == 0), stop=False,
                                 skip_group_check=True)
                nc.tensor.matmul(acc[:, 1, 0:HR], cI[cp][:], Tp[:, 1, :],
                                 start=(pe_acc_done == 0),
                                 stop=(pe_acc_done == n_pe_acc - 1),
                                 skip_group_check=True)
                pe_acc_done += 1

            if dbg:
                nc.sync.dma_start(d_dxqt.ap(), XQT[:])
                wslf_dbg = stage.tile([128, 4, NB, 128], f32, tag="wsldbg")
                nc.vector.tensor_copy(wslf_dbg[:], WSL[:])
                nc.sync.dma_start(d_dwsl.ap(), wslf_dbg[:])
                nc.sync.dma_start(d_dvrow.ap(), Vrow[:])
                accf_dbg = stage.tile([COUT, R], f32, tag="accdbg")
                accf_dbg3 = accf_dbg[:].rearrange("p (a n) -> p a n", a=2)
                nc.scalar.copy(accf_dbg3, acc[:, :, 0:HR])
                nc.sync.dma_start(d_dacc.ap(), accf_dbg[:])

            # ---------------- dequant + corrections ----------------
            # corr[cout, row] = U1'*xmin + U2*A_x + U3*xscale  (K=3 f32 matmul)
            cps = psz.tile([128, 2, 512], f32, tag="zps")
            nc.tensor.matmul(cps[:, 0, 0:HR], UT[:], Vrow[:, 0:HR],
                             start=True, stop=True)
            nc.tensor.matmul(cps[:, 1, 0:HR], UT[:], Vrow[:, HR:R],
                             start=True, stop=True)
            # xs broadcast tile via ones outer product
            xs_ps = psz.tile([128, 2, 512], f32, tag="zps")
            nc.tensor.matmul(xs_ps[:, 0, 0:HR], ones1[:], Vrow[0:1, 0:HR],
                             start=True, stop=True)
            nc.tensor.matmul(xs_ps[:, 1, 0:HR], ones1[:], Vrow[0:1, HR:R],
                             start=True, stop=True)

            outf = stage.tile([COUT, R], f32, tag="outf")
            outf3 = outf[:].rearrange("p (a n) -> p a n", a=2)
            if n_acc_d:
                # total = acc + out_dve; then scale and add corrections
                nc.vector.scalar_tensor_tensor(outf3, acc[:, :, 0:HR], 0.0,
                                               out_dve3, op0=OP.bypass, op1=OP.add)
                nc.vector.tensor_scalar(outf3, outf3, w_scale[:], None,
                                        op0=OP.mult)
            else:
                nc.vector.tensor_scalar(outf3, acc[:, :, 0:HR], w_scale[:], None,
                                        op0=OP.mult)
            nc.vector.scalar_tensor_tensor(outf3, outf3, 0.0, xs_ps[:, :, 0:HR],
                                           op0=OP.bypass, op1=OP.mult)
            nc.vector.tensor_tensor(outf3, outf3, cps[:, :, 0:HR], op=OP.add)
            nc.sync.dma_start(d_out.ap(), outf[:])

    nc.compile()
    return nc


def _get_nc():
    if "nc" not in _NC_CACHE:
        _NC_CACHE["nc"] = _build_program()
    return _NC_CACHE["nc"]


def _im2col_host(x):
    # 3x3 SAME patches, column order [Cin, kh, kw]; rows (b, h, w)
    xp = np.pad(x, ((0, 0), (0, 0), (1, 1), (1, 1)))  # [B, C, 58, 58]
    s = xp.strides
    v = np.lib.stride_tricks.as_strided(
        xp,
        shape=(B, H, W_, CIN, KH, KW),
        strides=(s[0], s[2], s[3], s[1], s[2], s[3]),
    )
    return v.reshape(NX, K)


def kernel(x, w):
    from concourse.bass_utils import run_bass_kernel_spmd

    nc = _get_nc()
    x = np.ascontiguousarray(np.asarray(x, dtype=np.float32))
    w = np.asarray(w, dtype=np.float32)

    xf = np.zeros((NX, KPAD), np.float32)
    xf[:, :K] = _im2col_host(x)
    wf = np.zeros((COUT, KPAD), np.float32)
    wf[:, :K] = w.reshape(COUT, K)

    in_maps = [{"xf": np.ascontiguousarray(xf[c * R:(c + 1) * R]), "wf": wf}
               for c in range(NCORES)]
    res = run_bass_kernel_spmd(nc, in_maps, core_ids=list(range(NCORES)))
    _NC_CACHE["last_results"] = res
    z = np.concatenate([res.results[c]["out"].T for c in range(NCORES)], axis=0)
    return np.ascontiguousarray(
        z.reshape(B, H, W_, COUT).transpose(0, 3, 1, 2).astype(np.float32))


# revision 13
# speedup vs baseline: 1.4123x; 1.0766x over previous
"""Trainium2 Bass kernel for nn_ConvDatapath: quantized bit-sliced crossbar conv.

v2 pipeline (per core, data-parallel over Nx=6272 im2col rows, 784 rows/core):
  host: im2col (layout only) -> xf [784, 580] per core (K 576 zero-padded to 580)
  device:
    prep (per 128-row tile j):
      - DVE min/max row reductions; Pool computes sum(x-min) via
        tensor_scalar accum_out; small per-row scale/bias ops
      - one ScalarE activation quantizes: qM = Relu(x*inv + (M - min*inv))
        (magic 2^23 add rounds to integer exactly)
      - ScalarE copies qM -> q as uint16 into a block-padded [128,5,128] tile
      - DMA-engine xbar transpose ([128,5,128] -> [128,5,128] per-block
        transposed) builds XQT [k, b, row] with zero engine cost
    slicing (per k-block b, pipelined with main loop):
      - 4 DVE u16 shift/and ops extract 2-bit planes (2x DVE mode)
      - 1 DVE copy converts planes to bf16 for the PE
    main loop (80 iterations = 5 blocks x 4 w-slices x 4 x-slices):
      - 2 bf16 matmuls [116]x[116,392] -> PSUM (exact small-int products)
      - ADC round: T = fp16(z*0.25 + 1024) on ScalarE or DVE; the f32->fp16
        convert rounds to integer exactly (values in [1024,2048) have ulp 1)
      - accumulate: PE matmul acc += (c*I)^T @ T into a persistent PSUM
        accumulator (fp16 x fp16 products exact in f32), a few on DVE
    tail: rank-3 correction matmul (offset terms + 1024-bias removal) and
      two DVE ops produce the dequantized output; DMA out [128, 784]
  host: gather per-core [128, 784] outputs -> [2,128,56,56]
"""
import sys

sys.path.insert(0, "/opt/trn_rl_repo")

import numpy as np

# ---- problem constants (hardcoded per contract) ----
B, CIN, H, W_ = 2, 64, 56, 56
COUT, KH, KW = 128, 3, 3
K = CIN * KH * KW            # 576
NB, NPB = 5, 116             # chunker: 5 blocks of 116 (pad 4)
KPAD = NB * NPB              # 580
NCORES = 8
NX = B * H * W_              # 6272
R = NX // NCORES             # 784 rows per core
RT = 128                     # row tile (xbar-aligned) -> 7 tiles, last has 16
NJ = 7
RPADT = NJ * RT              # 896 padded row count in transposed tiles
MAGIC = float(2 ** 23)
WSF = [64.0, 16.0, 4.0, 1.0]
ISF = [64.0, 16.0, 4.0, 1.0]
SH = [6, 4, 2, 0]            # slice shifts
HR = R // 2                  # 392
# c for iteration (ws, is); ADC out contributes c*round(z/4) with c=4*WSF*ISF
C_OF = [[4.0 * WSF[w] * ISF[i] for i in range(4)] for w in range(4)]
C_VALUES = sorted({C_OF[w][i] for w in range(4) for i in range(4)}, reverse=True)
C0 = 1024.0 * NB * sum(C_OF[w][i] for w in range(4) for i in range(4))

# scheduling knobs
ROUND_ON_S = [(i * 13) % 20 >= 7 for i in range(80)]  # ~52 ScalarE / 28 DVE
ACC_ON_D = [i % 8 == 5 for i in range(80)]       # 10 DVE / 70 PE accumulates

_NC_CACHE = {}


def _build_program():
    import concourse.bass as bass
    import concourse.bacc as bacc
    import concourse.tile as tile
    from concourse import mybir
    from concourse.masks import make_identity

    f32 = mybir.dt.float32
    u16 = mybir.dt.uint16
    bf16 = mybir.dt.bfloat16
    fp16 = mybir.dt.float16
    AF = mybir.ActivationFunctionType
    OP = mybir.AluOpType
    AX = mybir.AxisListType
    INV255 = float(np.float32(1.0 / 255.0))

    import os
    dbg = bool(os.environ.get("CONV_DEBUG"))
    nc = bacc.Bacc("TRN2", target_bir_lowering=False, debug=False)

    d_xf = nc.dram_tensor("xf", (R, KPAD), f32, kind="ExternalInput")
    d_wf = nc.dram_tensor("wf", (COUT, KPAD), f32, kind="ExternalInput")
    d_out = nc.dram_tensor("out", (COUT, R), f32, kind="ExternalOutput")
    if dbg:
        d_dxqt = nc.dram_tensor("dbg_xqt", (128, NJ, NB, 128), mybir.dt.uint16,
                                kind="ExternalOutput")
        d_dwsl = nc.dram_tensor("dbg_wsl", (128, 4, NB, 128), f32,
                                kind="ExternalOutput")
        d_dvrow = nc.dram_tensor("dbg_vrow", (3, RPADT), f32,
                                 kind="ExternalOutput")
        d_dacc = nc.dram_tensor("dbg_acc", (COUT, R), f32,
                                kind="ExternalOutput")

    with tile.TileContext(nc) as tc:
        with (
            tc.tile_pool(name="const", bufs=1) as cpool,
            tc.tile_pool(name="stage", bufs=3) as stage,
            tc.tile_pool(name="tpool", bufs=4) as tpool,
            tc.tile_pool(name="psz", bufs=3, space="PSUM") as psz,
            tc.tile_pool(name="pacc", bufs=1, space="PSUM") as pacc,
        ):
            ident = cpool.tile([128, 128], f32)
            make_identity(nc, ident[:])
            Mtile = cpool.tile([128, 1], f32)
            nc.vector.memset(Mtile[:], MAGIC)
            Kbias = cpool.tile([128, 1], f32)
            nc.vector.memset(Kbias[:], 1024.0)
            ones1 = cpool.tile([1, COUT], f32)
            nc.vector.memset(ones1[:], 1.0)

            # c * identity fp16 stationary tiles for the accumulate matmuls
            cI = {}
            for cv in C_VALUES:
                t = cpool.tile([128, 128], fp16, name=f"cI{int(cv)}")
                nc.gpsimd.memset(t[:], 0.0)
                nc.gpsimd.affine_select(out=t[:], in_=t[:],
                                        compare_op=OP.not_equal, fill=cv,
                                        base=0, pattern=[[-1, 128]],
                                        channel_multiplier=1)
                cI[cv] = t

            # ---------------- W prep ----------------
            w_sb = cpool.tile([COUT, KPAD], f32)
            nc.sync.dma_start(w_sb[:], d_wf.ap())
            w_min = cpool.tile([COUT, 1], f32)
            w_max = stage.tile([COUT, 1], f32, tag="wmax")
            nc.vector.tensor_reduce(w_min[:], w_sb[:], axis=AX.X, op=OP.min)
            nc.vector.tensor_reduce(w_max[:], w_sb[:], axis=AX.X, op=OP.max)
            w_scale = cpool.tile([COUT, 1], f32)
            # w_scale = (max - min)/255
            nc.vector.scalar_tensor_tensor(w_scale[:], w_max[:], 1.0, w_min[:],
                                           op0=OP.mult, op1=OP.subtract)
            nc.vector.tensor_scalar(w_scale[:], w_scale[:], INV255, None, op0=OP.mult)
            w_inv = cpool.tile([COUT, 1], f32)
            nc.vector.reciprocal(w_inv[:], w_scale[:])
            # vr = w - min (rounded f32, matching the reference), accum -> A_w
            w_vr = stage.tile([COUT, KPAD], f32, tag="wvr")
            w_accA = cpool.tile([COUT, 1], f32)
            nc.vector.tensor_scalar(w_vr[:], w_sb[:], w_min[:], 0.0,
                                    op0=OP.subtract, op1=OP.add,
                                    accum_out=w_accA[:])
            # quantize: qMw = Relu(vr*inv + M) = M + qw
            qMw = stage.tile([COUT, KPAD], f32, tag="qMw")
            nc.scalar.activation(qMw[:], w_vr[:], AF.Relu, bias=Mtile[:],
                                 scale=w_inv[:])
            # u16ify into block-padded layout
            wq_u16 = stage.tile([COUT, NB, 128], u16, tag="wqu")
            nc.scalar.activation(
                wq_u16[:, :, 0:NPB],
                qMw[:].rearrange("p (b c) -> p b c", b=NB),
                AF.Copy, bias=-MAGIC)
            # xbar transpose: WQT[k, b, cout]
            WQT = cpool.tile([128, NB, 128], u16)
            nc.sync.dma_start_transpose(
                WQT[:], wq_u16[:].rearrange("p b c -> p (b c)"))
            # slices: WSLu[k, s, b, cout] -> bf16
            WSLu = stage.tile([128, 4, NB, 128], u16, tag="wslu")
            for s in range(4):
                if SH[s]:
                    nc.vector.tensor_scalar(WSLu[:, s], WQT[:], SH[s], 3,
                                            op0=OP.logical_shift_right,
                                            op1=OP.bitwise_and)
                else:
                    nc.vector.tensor_scalar(WSLu[:, s], WQT[:], 3, None,
                                            op0=OP.bitwise_and)
            WSL = cpool.tile([128, 4, NB, 128], bf16)
            nc.vector.tensor_copy(WSL[:], WSLu[:])

            # correction row operands:
            # U1' = A_w + 580*w_min (pairs x_min)
            # U2  = w_min           (pairs A_x = sum(x-min) over 580)
            # U3  = -C0*w_scale     (pairs x_scale; removes 1024 round bias)
            Upair = stage.tile([COUT, 3], f32, tag="upair")
            nc.vector.tensor_scalar(Upair[:, 0:1], w_scale[:], -C0, None,
                                    op0=OP.mult)
            nc.vector.scalar_tensor_tensor(Upair[:, 1:2], w_min[:], 584.0,
                                           w_accA[:], op0=OP.mult, op1=OP.add)
            nc.vector.tensor_copy(Upair[:, 2:3], w_min[:])
            ps_u = psz.tile([128, 2, 512], f32, tag="zps")
            nc.tensor.transpose(ps_u[0:3, 0, 0:COUT], Upair[:], ident[:])
            UT = cpool.tile([3, COUT], f32)
            nc.scalar.copy(UT[:], ps_u[0:3, 0, 0:COUT])

            # ---------------- X prep ----------------
            XQT = cpool.tile([128, NJ, NB, 128], u16)   # [k, j, b, row_in_j]
            Vrow = cpool.tile([3, RPADT], f32)          # rows: xscale, xmin, A_x

            for j in range(NJ):
                nr = RT if j < NJ - 1 else R - RT * (NJ - 1)   # 128 or 16
                x_sb = stage.tile([RT, KPAD], f32, tag="x_sb")
                nc.sync.dma_start(x_sb[0:nr, :], d_xf.ap()[j * RT:j * RT + nr, :])
                Vtri = stage.tile([RT, 3], f32, tag="vtri")
                xmax = stage.tile([RT, 1], f32, tag="xmax")
                nc.vector.tensor_reduce(Vtri[:, 1:2], x_sb[:], axis=AX.X, op=OP.min)
                nc.vector.tensor_reduce(xmax[:], x_sb[:], axis=AX.X, op=OP.max)
                # xscale = (max-min)/255 into Vtri col 0 (Pool)
                nc.gpsimd.tensor_tensor(Vtri[:, 0:1], xmax[:], Vtri[:, 1:2],
                                        op=OP.subtract)
                nc.gpsimd.tensor_scalar(Vtri[:, 0:1], Vtri[:, 0:1], INV255, None,
                                        op0=OP.mult)
                xinv = stage.tile([RT, 1], f32, tag="xinv")
                nc.vector.reciprocal(xinv[:], Vtri[:, 0:1])
                # vr = x - min (rounded f32, matching reference), accum -> A_x
                vr_scr = stage.tile([RT, KPAD], f32, tag="vrscr")
                nc.vector.tensor_scalar(vr_scr[:], x_sb[:], Vtri[:, 1:2], 0.0,
                                        op0=OP.subtract, op1=OP.add,
                                        accum_out=Vtri[:, 2:3])
                # quantize
                qMx = stage.tile([RT, KPAD], f32, tag="qMx")
                nc.scalar.activation(qMx[:], vr_scr[:], AF.Relu, bias=Mtile[0:RT],
                                     scale=xinv[:])
                # u16ify block-padded
                qu16 = stage.tile([RT, NB, 128], u16, tag="qu16")
                nc.scalar.activation(
                    qu16[:, :, 0:NPB],
                    qMx[:].rearrange("p (b c) -> p b c", b=NB),
                    AF.Copy, bias=-MAGIC)
                # zero k=576..579 pad cols of block 4 before transposing
                nc.vector.memset(qu16[:, 4, 112:116], 0)
                # xbar transpose into XQT columns j*128..+128
                nc.sync.dma_start_transpose(
                    XQT[:, j],
                    qu16[:].rearrange("p b c -> p (b c)"))
                # stats transpose -> Vrow cols
                ps_v = psz.tile([128, 2, 512], f32, tag="zps")
                nc.tensor.transpose(ps_v[0:3, 0, 0:RT], Vtri[:], ident[0:RT, 0:RT])
                nc.scalar.copy(Vrow[:, j * RT:(j + 1) * RT], ps_v[0:3, 0, 0:RT])


            # ---------------- main loop with inline slicing ----------------
            acc = pacc.tile([128, 2, 512], f32)
            out_dve = cpool.tile([COUT, R], f32)
            out_dve3 = out_dve[:].rearrange("p (a n) -> p a n", a=2)
            n_acc_d = sum(1 for i in range(80) if ACC_ON_D[i])

            def emit_slice(b):
                xslu = tpool.tile([128, 4, RPADT], u16, tag="xslu")
                for s in range(4):
                    if SH[s]:
                        nc.vector.tensor_scalar(xslu[:, s], XQT[:, :, b, :],
                                                SH[s], 3,
                                                op0=OP.logical_shift_right,
                                                op1=OP.bitwise_and)
                    else:
                        nc.vector.tensor_scalar(xslu[:, s], XQT[:, :, b, :],
                                                3, None, op0=OP.bitwise_and)
                xsl = tpool.tile([128, 4, RPADT], bf16, tag="xsl")
                nc.vector.tensor_copy(xsl[:], xslu[:])
                return xsl

            xsl_cur = emit_slice(0)
            xsl_next = None
            pend = []           # (kind, payload) pending PE accumulate emission
            it = 0
            first_d = True
            n_pe_acc = 80 - n_acc_d
            pe_acc_done = 0
            for b in range(NB):
                for wsi in range(4):
                    for isi in range(4):
                        c = C_OF[wsi][isi]
                        zps = psz.tile([128, 2, 512], f32, tag="zps")
                        nc.tensor.matmul(zps[:, 0, 0:HR],
                                         WSL[0:NPB, wsi, b, :],
                                         xsl_cur[0:NPB, isi, 0:HR],
                                         start=True, stop=True)
                        nc.tensor.matmul(zps[:, 1, 0:HR],
                                         WSL[0:NPB, wsi, b, :],
                                         xsl_cur[0:NPB, isi, HR:R],
                                         start=True, stop=True)
                        # flush one pending accumulate (software pipeline lag 2)
                        if len(pend) >= 2:
                            Tp, cp = pend.pop(0)
                            nc.tensor.matmul(acc[:, 0, 0:HR], cI[cp][:],
                                             Tp[:, 0, :], start=(pe_acc_done == 0),
                                             stop=False, skip_group_check=True)
                            nc.tensor.matmul(acc[:, 1, 0:HR], cI[cp][:],
                                             Tp[:, 1, :], start=(pe_acc_done == 0),
                                             stop=(pe_acc_done == n_pe_acc - 1),
                                             skip_group_check=True)
                            pe_acc_done += 1
                        # ADC round
                        T = tpool.tile([128, 2, HR], fp16, tag="T")
                        if ROUND_ON_S[it]:
                            nc.scalar.activation(T[:], zps[:, :, 0:HR], AF.Relu,
                                                 bias=Kbias[:], scale=0.25)
                        else:
                            nc.vector.tensor_scalar(T[:], zps[:, :, 0:HR],
                                                    0.25, 1024.0,
                                                    op0=OP.mult, op1=OP.add)
                        # accumulate
                        if ACC_ON_D[it]:
                            if first_d:
                                nc.vector.tensor_scalar(out_dve3, T[:], c, None,
                                                        op0=OP.mult)
                                first_d = False
                            else:
                                nc.vector.scalar_tensor_tensor(out_dve3, T[:], c,
                                                               out_dve3,
                                                               op0=OP.mult,
                                                               op1=OP.add)
                        else:
                            pend.append((T, c))
                        # interleave next block's slicing mid-block
                        if isi == 3 and wsi == 1 and b < NB - 1:
                            xsl_next = emit_slice(b + 1)
                        it += 1
                xsl_cur = xsl_next
            # flush remaining pending accumulates
            while pend:
                Tp, cp = pend.pop(0)
                nc.tensor.matmul(acc[:, 0, 0:HR], cI[cp][:], Tp[:, 0, :],
                                 start=(pe_acc_done == 0), stop=False,
                                 skip_group_check=True)
                nc.tensor.matmul(acc[:, 1, 0:HR], cI[cp][:], Tp[:, 1, :],
                                 start=(pe_acc_done == 0),
                                 stop=(pe_acc_done == n_pe_acc - 1),
                                 skip_group_check=True)
                pe_acc_done += 1

            if dbg:
                nc.sync.dma_start(d_dxqt.ap(), XQT[:])
                wslf_dbg = stage.tile([128, 4, NB, 128], f32, tag="wsldbg")
                nc.vector.tensor_copy(wslf_dbg[:], WSL[:])
                nc.sync.dma_start(d_dwsl.ap(), wslf_dbg[:])
                nc.sync.dma_start(d_dvrow.ap(), Vrow[:])
                accf_dbg = stage.tile([COUT, R], f32, tag="accdbg")
                accf_dbg3 = accf_dbg[:].rearrange("p (a n) -> p a n", a=2)
                nc.scalar.copy(accf_dbg3, acc[:, :, 0:HR])
                nc.sync.dma_start(d_dacc.ap(), accf_dbg[:])

            # ---------------- dequant + corrections ----------------
            # corr[cout, row] = U1'*xmin + U2*A_x + U3*xscale  (K=3 f32 matmul)
            cps = psz.tile([128, 2, 512], f32, tag="zps")
            nc.tensor.matmul(cps[:, 0, 0:HR], UT[:], Vrow[:, 0:HR],
                             start=True, stop=True)
            nc.tensor.matmul(cps[:, 1, 0:HR], UT[:], Vrow[:, HR:R],
                             start=True, stop=True)
            # xs broadcast tile via ones outer product
            xs_ps = psz.tile([128, 2, 512], f32, tag="zps")
            nc.tensor.matmul(xs_ps[:, 0, 0:HR], ones1[:], Vrow[0:1, 0:HR],
                             start=True, stop=True)
            nc.tensor.matmul(xs_ps[:, 1, 0:HR], ones1[:], Vrow[0:1, HR:R],
                             start=True, stop=True)

            outf = stage.tile([COUT, R], f32, tag="outf")
            outf3 = outf[:].rearrange("p (a n) -> p a n", a=2)
            if n_acc_d:
                # total = acc + out_dve; then scale and add corrections
                nc.vector.scalar_tensor_tensor(outf3, acc[:, :, 0:HR], 0.0,
                                               out_dve3, op0=OP.bypass, op1=OP.add)
                nc.vector.tensor_scalar(outf3, outf3, w_scale[:], None,
                                        op0=OP.mult)
            else:
                nc.vector.tensor_scalar(outf3, acc[:, :, 0:HR], w_scale[:], None,
                                        op0=OP.mult)
            nc.vector.scalar_tensor_tensor(outf3, outf3, 0.0, xs_ps[:, :, 0:HR],
                                           op0=OP.bypass, op1=OP.mult)
            nc.vector.tensor_tensor(outf3, outf3, cps[:, :, 0:HR], op=OP.add)
            nc.sync.dma_start(d_out.ap(), outf[:])

    nc.compile()
    return nc


def _get_nc():
    if "nc" not in _NC_CACHE:
        _NC_CACHE["nc"] = _build_program()
    return _NC_CACHE["nc"]


def _im2col_host(x):
    # 3x3 SAME patches, column order [Cin, kh, kw]; rows (b, h, w)
    xp = np.pad(x, ((0, 0), (0, 0), (1, 1), (1, 1)))  # [B, C, 58, 58]
    s = xp.strides
    v = np.lib.stride_tricks.as_strided(
        xp,
        shape=(B, H, W_, CIN, KH, KW),
        strides=(s[0], s[2], s[3], s[1], s[2], s[3]),
    )
    return v.reshape(NX, K)


def kernel(x, w):
    from concourse.bass_utils import run_bass_kernel_spmd

    nc = _get_nc()
    x = np.ascontiguousarray(np.asarray(x, dtype=np.float32))
    w = np.asarray(w, dtype=np.float32)

    xf = np.zeros((NX, KPAD), np.float32)
    xf[:, :K] = _im2col_host(x)
    wf = np.zeros((COUT, KPAD), np.float32)
    wf[:, :K] = w.reshape(COUT, K)

    in_maps = [{"xf": np.ascontiguousarray(xf[c * R:(c + 1) * R]), "wf": wf}
               for c in range(NCORES)]
    res = run_bass_kernel_spmd(nc, in_maps, core_ids=list(range(NCORES)))
    _NC_CACHE["last_results"] = res
    z = np.concatenate([res.results[c]["out"].T for c in range(NCORES)], axis=0)
    return np.ascontiguousarray(
        z.reshape(B, H, W_, COUT).transpose(0, 3, 1, 2).astype(np.float32))
